# revision 87
# baseline (speedup 1.0000x reference)
"""Trainium2 Bass kernel for nn_LunaCausalAttention.

Sharding: 8 cores; core c handles batch b = c//4 and heads hs = 4*(c%4) .. hs+4.
Per core: feature-major projections (bf16 matmuls), chunked two-pass causal
linear attention (C=128, head pairs packed into the 128-partition dim), and a
partial output projection over its 256 head-features. Host sums the 4 bf16
partials per batch and adds the output bias.

Key design points (115302ns baseline -> 73071ns):
- pq stored block-diagonal per head-pair: pattn (Z) is ONE K=128 matmul, one
  exp and one ln per PAIR OF HEAD-PAIRS (all elementwise ops process both
  head-pairs in one wide op -- engine time is free-dim-size driven, the
  partition dim is free, and fixed op overheads are 60-185ns).
- Chunk work is a software pipeline: stageA(c+2) (Z/softplus/transposes/G)
  runs ahead; stageB is split around the softmax so stageA's independent
  DVE work interleaves into stageB's dependency gaps; out-blocks trail by
  two chunks to fill the pipeline drain.
- PSUM banks are tagged by consumer-timing (aA/aB for stageA, parity b{01}x
  for stageB, wM, pp) because a bank-tile request WAR-waits on ALL readers
  of the bank's previous tile.
- Engine balance: Act = exp/ln/copies, DVE = masks/state-adds/reduce,
  Pool = pt2 scaling only (GPSIMD cannot touch PSUM on this HW).
- Output path: out matmul -> PSUM -> f32->bf16 copies (Act+DVE) -> bf16 DMA
  per half; out bias added on host during the partial-sum reduce.
- DMA: ~625ns serial HWDGE issue per dma_start, 360GB/s serial transfer ->
  few large host-prearranged transfers (>=1KB/partition contiguous), with
  w0/xta in per-f granules so projections chase arrivals; nh1 (tokens
  512:1024) loads ride behind the chunk-pipeline start.
"""
import numpy as np

import concourse.bass as bass
import concourse.mybir as mybir
import concourse.tile as tile
from concourse import bacc
from concourse.masks import make_upper_triangular, make_identity
from concourse.bass_utils import run_bass_kernel_spmd

# static shapes
B, N, D, M, H, DH = 2, 1024, 1024, 64, 16, 64
C = 128                 # token chunk
NCH = N // C            # 8 chunks
NCORES = 8
HPC = 4                 # heads per core
E = HPC * DH            # 256 per-core head features
NF = D // 128           # 8 contraction tiles
BETA = float(np.log(2.0))
SCALE = DH ** -0.5

F32 = mybir.dt.float32
BF16 = mybir.dt.bfloat16
ADT = BF16              # attention-core operand dtype
AF = mybir.ActivationFunctionType


def build_bass(phase=3):
    nc = bacc.Bacc(None, target_bir_lowering=False)

    # ---- I/O (all host-prearranged layouts; see make_in_maps) ----
    # xt half-tiles: xta = tokens 0:512, xtb = 512:1024, each [128, NF*512]
    xta_d = nc.dram_tensor("xta", [128, NF, 512], BF16, kind="ExternalInput")
    xtb_d = nc.dram_tensor("xtb", [128, NF, 512], BF16, kind="ExternalInput")
    pT_d = nc.dram_tensor("pT", [128, NF + 4, M], BF16, kind="ExternalInput")
    wpq_d = nc.dram_tensor("wpq", [128, NF, E], BF16, kind="ExternalInput")  # scale folded
    wv_d = nc.dram_tensor("wv", [128, NF, E], BF16, kind="ExternalInput")
    w0_d = nc.dram_tensor("w0", [128, NF, 3, 128], BF16, kind="ExternalInput")  # q,k,pc et0
    w1_d = nc.dram_tensor("w1", [128, NF, 3, 128], BF16, kind="ExternalInput")  # q,k,pc et1
    wo_d = nc.dram_tensor("wo", [128, 2, D], BF16, kind="ExternalInput")
    # aux packs bq|bk|bpc|bpq (2 cols each), rc (8), bvb (256) -- ONE DMA
    # (every dma_start costs ~625ns of serial HWDGE issue time)
    aux_d = nc.dram_tensor("aux", [128, 272], F32, kind="ExternalInput")
    out_d = nc.dram_tensor("outp", [N, D], BF16, kind="ExternalOutput")

    with tile.TileContext(nc) as tc:
        with (
            tc.tile_pool(name="singles", bufs=1) as singles,
            tc.tile_pool(name="work", bufs=6) as work,
            tc.tile_pool(name="obuf", bufs=3) as obuf,
            tc.tile_pool(name="psum", bufs=1, space="PSUM") as psum,
        ):
            # ---- constants ----
            triu = singles.tile([128, 2 * C], F32)      # two upper-tri copies
            make_upper_triangular(nc, triu[:, 0:C], val=1.0, diag=True)
            make_upper_triangular(nc, triu[:, C:2 * C], val=1.0, diag=True)
            identb = singles.tile([128, 128], ADT)
            make_identity(nc, identb)

            # ---- DMA in earliest-need order (few, large transfers) ----
            ones_row = singles.tile([1, 128], BF16)
            nc.vector.memset(ones_row, 1.0)
            # w0/xta arrive in interleaved per-f granules so the first
            # projection matmuls start as early as possible
            GRAN = [(0, 1), (1, 2), (2, 4), (4, 8)]

            def granule_tiles(dram, inner, nm):
                tiles = [singles.tile([128, g1 - g0] + inner, BF16,
                                      name=f"{nm}_{g0}")
                         for g0, g1 in GRAN]
                def view(f):
                    for (g0, g1), tl in zip(GRAN, tiles):
                        if g0 <= f < g1:
                            return tl[:, f - g0]
                return tiles, view

            w0_tiles, w0v = granule_tiles(w0_d, [3, 128], "w0g")
            xta_tiles, xta = granule_tiles(xta_d, [512], "xtag")
            for (g0, g1), wt, xt_ in zip(GRAN, w0_tiles, xta_tiles):
                nc.sync.dma_start(out=wt, in_=w0_d[:, g0:g1])
                nc.sync.dma_start(out=xt_, in_=xta_d[:, g0:g1])
            aux_sb = singles.tile([128, 272], F32, name="aux_sb")
            nc.sync.dma_start(out=aux_sb, in_=aux_d[:, :])
            bq_sb = aux_sb[:, 0:2]
            bk_sb = aux_sb[:, 2:4]
            bpc_sb = aux_sb[:, 4:6]
            bpq_sb = aux_sb[:, 6:8]
            rc_sb = aux_sb[:, 8:16]
            wpq_sb = singles.tile([128, NF, E], BF16, name="wpq_sb")
            nc.sync.dma_start(out=wpq_sb, in_=wpq_d[:, :, :])
            pT_sb = singles.tile([128, NF + 4, M], BF16)
            nc.sync.dma_start(out=pT_sb, in_=pT_d[:, :, :])
            bv_row = pT_sb[0:1, NF:NF + 4, :]   # (1, 256) bv on partition 0
            w1_t = [singles.tile([128, 4, 3, 128], BF16, name=f"w1t{i}")
                    for i in range(2)]
            for i in range(2):
                nc.sync.dma_start(out=w1_t[i], in_=w1_d[:, 4 * i:4 * i + 4])
            wv_sb = singles.tile([128, NF, E], BF16, name="wv_sb")
            nc.sync.dma_start(out=wv_sb, in_=wv_d[:, :, :])
            xtb_t = [singles.tile([128, 4, 512], BF16, name=f"xtb{i}")
                     for i in range(2)]
            for i in range(2):
                nc.sync.dma_start(out=xtb_t[i], in_=xtb_d[:, 4 * i:4 * i + 4, :])
            xtb = lambda f: xtb_t[f // 4][:, f % 4]
            wo_sb = singles.tile([128, 2, D], BF16, name="wo_sb")
            nc.sync.dma_start(out=wo_sb, in_=wo_d[:, :, :])

            def xt(f, nh):  # (128, 512) token-half view
                return (xta if nh == 0 else xtb)(f)

            def xtc(f, c):  # (128, 128) chunk-of-tokens view
                return xt(f, c // 4)[:, (c % 4) * 128:(c % 4) * 128 + 128]

            # projection outputs
            qT_sb = singles.tile([128, 2, N], ADT)    # [:, et, t] feature-major
            kT_sb = singles.tile([128, 2, N], ADT)
            pcT_sb = singles.tile([128, 2, N], ADT)
            pq_bd = singles.tile([128, 2, 128], ADT)  # block-diag pq per pair
            vtok_sb = [singles.tile([128, E], ADT, name=f"vtok{t}")
                       for t in range(NCH)]
            attnT_sb = [singles.tile([128, 2, C], ADT, name=f"attnT{t}")
                        for t in range(NCH)]
            # state cols: [S hp0 | S hp1 | T hp0 | T hp1] (64 each; T rows
            # blockdiag). S and T update as one wide DVE add each; S updates
            # early (it gates the next chunk's aw), T after its last reader.
            ST_sb = singles.tile([128, 256], ADT)
            nc.vector.memset(ST_sb, 0.0)
            nc.gpsimd.memset(pq_bd, 0.0)

            # ---- pq projection into block-diagonal tiles ----
            def pq_proj():
                for hp in range(2):
                    ppq = psum.tile([128, 512], F32, tag="pp", bufs=1,
                                    name="ppq")
                    for f in range(NF):
                        nc.tensor.matmul(
                            ppq[:, 0:M], wpq_sb[:, f, hp * 128:(hp + 1) * 128],
                            pT_sb[:, f, :],
                            start=(f == 0), stop=(f == NF - 1))
                    for h in range(2):
                        nc.vector.tensor_scalar_add(
                            pq_bd[64 * h:64 * h + 64, hp, 64 * h:64 * h + 64],
                            ppq[64 * h:64 * h + 64, 0:M],
                            bpq_sb[64 * h:64 * h + 64, hp:hp + 1])

            # ---- feature-major projections: qT, kT, pcT ----
            # q/k/pc interleaved per f-tile (3 open psum groups in separate
            # banks) so the first matmul runs as soon as xt[f=0] lands;
            # PSUM->SBUF copies (with bias) split across DVE and Act.
            PROJ_TAGS = ["pp", "aA", "aB", "b00", "b01", "b10", "b11",
                         "wM"]

            def proj_group(et, nh):
                def w(f):
                    return w0v(f) if et == 0 else w1_t[f // 4][:, f % 4]
                g = 3 * (2 * et + nh)
                pps = [psum.tile([128, 512], F32,
                                 tag=PROJ_TAGS[(g + i) % 8], bufs=1,
                                 name=f"pj{i}")
                       for i in range(3)]
                for f in range(NF):
                    for wi in range(3):
                        nc.tensor.matmul(
                            pps[wi], w(f)[:, wi], xt(f, nh),
                            start=(f == 0), stop=(f == NF - 1))
                cols = slice(nh * 512, (nh + 1) * 512)
                nc.vector.tensor_scalar_add(
                    qT_sb[:, et, cols], pps[0], bq_sb[:, et:et + 1])
                nc.vector.tensor_scalar_add(
                    kT_sb[:, et, cols], pps[1], bk_sb[:, et:et + 1])
                nc.vector.tensor_scalar_add(
                    pcT_sb[:, et, cols], pps[2], bpc_sb[:, et:et + 1])

            proj_group(0, 0)
            pq_proj()
            proj_group(1, 0)

            # ---- token-major projection: V_tok (bias via broadcast add) ----
            def vtok_proj(tb):
                pkv = psum.tile([128, 512], F32, tag="pp", bufs=1, name="pkv")
                for f in range(NF):
                    nc.tensor.matmul(
                        pkv[:, 0:E], xtc(f, tb), wv_sb[:, f, :],
                        start=(f == 0), stop=False)
                nc.tensor.matmul(pkv[:, 0:E], ones_row, bv_row,
                                 start=False, stop=True)
                nc.scalar.activation(vtok_sb[tb], pkv[:, 0:E], AF.Copy)

            # ---- attention chunks: 2-stage software pipeline, both head
            # pairs fused per stage so elementwise ops are wide (fixed op
            # overheads amortize; engines see fewer, bigger ops) ----
            lo, hi = slice(0, 64), slice(64, 128)
            sls = (lo, hi)
            st = {}
            bst = {}

            def stageA(c):
                tok = slice(c * C, (c + 1) * C)
                pA = psum.tile([128, 512], F32, tag="aA", bufs=1, name="pA")
                pB = psum.tile([128, 512], F32, tag="aB", bufs=1, name="pB")
                pz2 = pA[:, 256:512]                        # Z hp0|hp1
                pka = pB[:, 256:512].bitcast(ADT)           # (128,512) bf16
                pgs = (pA[:, 0:256], pB[:, 0:256])          # G h0|h1 banks

                # Z (pattn token-major) via block-diag pq: one matmul per hp
                for hp in (0, 1):
                    nc.tensor.matmul(pz2[:, 128 * hp:128 * hp + 128],
                                     pcT_sb[:, hp, tok], pq_bd[:, hp, :],
                                     start=True, stop=True)
                ez2 = work.tile([128, 256], F32, name="ez2", bufs=4)
                nc.scalar.activation(ez2, pz2, AF.Exp, scale=BETA)
                z2 = work.tile([128, 256], ADT, name="z2", bufs=4)
                nc.scalar.activation(z2, ez2, AF.Ln, bias=1.0, scale=1.0)

                # transposes: [K_tok hp0 | z^T hp0 | K_tok hp1 | z^T hp1]
                # then ONE merged PSUM->SBUF copy on Act
                for hp in (0, 1):
                    nc.tensor.transpose(pka[:, 256 * hp:256 * hp + 128],
                                        kT_sb[:, hp, tok], identb)
                    nc.tensor.transpose(pka[:, 256 * hp + 128:256 * hp + 256],
                                        z2[:, 128 * hp:128 * hp + 128],
                                        identb)
                ka2 = work.tile([128, 512], ADT, name="ka2", bufs=4)
                nc.scalar.activation(ka2, pka, AF.Copy)

                # G^T = K_c Q_c^T (j,i): h -> bank, hp -> cols; one wide
                # mask multiply per bank (DVE)
                gm2 = work.tile([128, 512], ADT, name="gm2", bufs=4)
                for hp in (0, 1):
                    for h in (0, 1):
                        s = sls[h]
                        nc.tensor.matmul(
                            pgs[h][:, 128 * hp:128 * hp + 128],
                            kT_sb[s, hp, tok], qT_sb[s, hp, tok],
                            start=True, stop=True, tile_position=(64 * h, 0))
                for h in (0, 1):
                    nc.vector.tensor_mul(gm2[:, 256 * h:256 * h + 256],
                                         pgs[h], triu[:, 0:2 * C])
                st[c] = (ka2, z2, gm2)

            def stageB(c):
                tok = slice(c * C, (c + 1) * C)
                ka2, z2, gm2 = st.pop(c)
                w0 = psum.tile([128, 512], F32, tag=f"b{c % 2}0", bufs=1,
                               name="w0")
                w1 = psum.tile([128, 512], F32, tag=f"b{c % 2}1", bufs=1,
                               name="w1")
                wM = psum.tile([128, 512], F32, tag="wM", bufs=1, name="wM")
                paw = (w0[:, 0:128], w1[:, 0:128])          # aw h -> bank
                pg2 = (w0[:, 128:384], w1[:, 128:384])      # G2 h -> bank
                psd = w0[:, 384:512]                        # S upd [hp0|hp1]
                ptd = w1[:, 384:512]                        # T upd [hp0|hp1]
                pptT = wM[:, 0:128].bitcast(ADT)            # (128,256) bf16
                pan = wM[:, 128:384]                        # attn^T hp0|hp1

                def zh(hp, h):
                    return z2[:, 128 * hp + 64 * h:128 * hp + 64 * h + 64]

                def ktc(hp, h):
                    return ka2[:, 256 * hp + 64 * h:256 * hp + 64 * h + 64]

                def at(hp, s):
                    return ka2[s, 256 * hp + 128:256 * hp + 256]

                # aw = Gm^T Z (+ Q S); h-major so ex/reduce of h0 overlap
                # the h1 matmuls
                for h in (0, 1):
                    for hp in (0, 1):
                        nc.tensor.matmul(
                            paw[h][:, 64 * hp:64 * hp + 64],
                            gm2[:, 256 * h + 128 * hp:256 * h + 128 * hp + 128],
                            zh(hp, h), start=True, stop=(c == 0))
                    if c > 0:
                        s = sls[h]
                        for hp in (0, 1):
                            nc.tensor.matmul(
                                paw[h][:, 64 * hp:64 * hp + 64],
                                qT_sb[s, hp, tok], ST_sb[s, 64 * hp:64 * hp + 64],
                                start=False, stop=True,
                                tile_position=(64 * h, 0))

                # S update EARLY (gates next chunk's aw); T's last reader is
                # the attn matmul below, so its update comes after that
                for hp in (0, 1):
                    for h in (0, 1):
                        nc.tensor.matmul(
                            psd[64 * h:64 * h + 64, 64 * hp:64 * hp + 64],
                            ktc(hp, h), zh(hp, h),
                            start=True, stop=True,
                            tile_position=(0, 64 * h))
                nc.vector.tensor_add(ST_sb[:, 0:128], ST_sb[:, 0:128], psd)

                # softmax (scales folded): P~ = exp(rc*aw)*rc/sum
                # ex2 cols: [h0: hp0|hp1 | h1: hp0|hp1]
                ex2 = work.tile([128, 256], F32, name="ex2")
                rs = work.tile([128, 4], F32, name="rs")
                for h in (0, 1):
                    nc.scalar.activation(ex2[:, 128 * h:128 * h + 128],
                                         paw[h], AF.Exp,
                                         scale=rc_sb[:, c:c + 1])
                    nc.vector.tensor_reduce(
                        rs[:, 2 * h:2 * h + 2],
                        ex2[:, 128 * h:128 * h + 128].rearrange(
                            "p (g m) -> p g m", g=2),
                        mybir.AxisListType.X, mybir.AluOpType.add)
                rcp = work.tile([128, 4], F32, name="rcp")
                nc.vector.reciprocal(rcp, rs)
                # pt2 cols: [hp0: h0|h1 | hp1: h0|h1] (m2-major per hp)
                pt2 = work.tile([128, 256], ADT, name="pt2")
                for hp in (0, 1):
                    eng = nc.gpsimd if hp == 0 else nc.vector
                    for h in (0, 1):
                        eng.tensor_scalar(
                            pt2[:, 128 * hp + 64 * h:128 * hp + 64 * h + 64],
                            ex2[:, 128 * h + 64 * hp:128 * h + 64 * hp + 64],
                            rcp[:, 2 * h + hp:2 * h + hp + 1],
                            rc_sb[:, c:c + 1], mybir.AluOpType.mult,
                            mybir.AluOpType.mult)

                # P~^T per hp; ONE merged copy on Act
                for hp in (0, 1):
                    nc.tensor.transpose(pptT[:, 128 * hp:128 * hp + 128],
                                        pt2[:, 128 * hp:128 * hp + 128],
                                        identb)
                ptT2 = work.tile([128, 256], ADT, name="ptT2")
                nc.scalar.activation(ptT2, pptT, AF.Copy)
                bst[c] = (ka2, z2, w0, w1, wM, ptT2)

            def stageB2(c):
                tok = slice(c * C, (c + 1) * C)
                ka2, z2, w0, w1, wM, ptT2 = bst.pop(c)
                pg2 = (w0[:, 128:384], w1[:, 128:384])
                ptd = w1[:, 384:512]
                pan = wM[:, 128:384]

                def zh(hp, h):
                    return z2[:, 128 * hp + 64 * h:128 * hp + 64 * h + 64]

                def at(hp, s):
                    return ka2[s, 256 * hp + 128:256 * hp + 256]

                # pass 2: G2^T = Z_c P~^T; h -> bank; wide masks (DVE)
                g2m2 = work.tile([128, 512], ADT, name="g2m2")
                for hp in (0, 1):
                    for h in (0, 1):
                        s = sls[h]
                        nc.tensor.matmul(
                            pg2[h][:, 128 * hp:128 * hp + 128],
                            at(hp, s), ptT2[s, 128 * hp:128 * hp + 128],
                            start=True, stop=True, tile_position=(64 * h, 0))
                for h in (0, 1):
                    nc.vector.tensor_mul(g2m2[:, 256 * h:256 * h + 256],
                                         pg2[h], triu[:, 0:2 * C])

                # attn^T = V^T G2m (+ T^T P~^T); hp -> col half, then ONE
                # wide Act copy for both head pairs
                for h in (0, 1):
                    s = sls[h]
                    for hp in (0, 1):
                        ph = pan[:, 128 * hp:128 * hp + 128]
                        vhh = vtok_sb[c][:, hp * 128 + 64 * h:
                                         hp * 128 + 64 * h + 64]
                        nc.tensor.matmul(
                            ph[64 * h:64 * h + 64, :], vhh,
                            g2m2[:, 256 * h + 128 * hp:256 * h + 128 * hp + 128],
                            start=True, stop=(c == 0),
                            tile_position=(0, 64 * h))
                        if c > 0:
                            nc.tensor.matmul(
                                ph[64 * h:64 * h + 64, :],
                                ST_sb[s, 128 + 64 * hp:128 + 64 * hp + 64],
                                ptT2[s, 128 * hp:128 * hp + 128],
                                start=False, stop=True,
                                tile_position=(64 * h, 64 * h))
                nc.scalar.activation(
                    attnT_sb[c].rearrange("p hp t -> p (hp t)"), pan, AF.Copy)

                # T update (after its last reader above)
                for hp in (0, 1):
                    vh = (vtok_sb[c][:, hp * 128:hp * 128 + 64],
                          vtok_sb[c][:, hp * 128 + 64:hp * 128 + 128])
                    for h in (0, 1):
                        nc.tensor.matmul(
                            ptd[64 * h:64 * h + 64, 64 * hp:64 * hp + 64],
                            zh(hp, h), vh[h],
                            start=True, stop=True,
                            tile_position=(0, 64 * h))
                nc.vector.tensor_add(ST_sb[:, 128:256], ST_sb[:, 128:256], ptd)

            def out_block(c):
                tok = slice(c * C, (c + 1) * C)
                ob = obuf.tile([128, D], BF16, name="ob")
                for oh in range(2):
                    tg = "aA" if (oh == 1 and c >= 6) else "pp"
                    po = psum.tile([128, 512], F32, tag=tg, bufs=1,
                                   name="po")
                    for et in range(2):
                        nc.tensor.matmul(
                            po, attnT_sb[c][:, et, :],
                            wo_sb[:, et, oh * 512:(oh + 1) * 512],
                            start=(et == 0), stop=(et == 1))
                    half = ob[:, oh * 512:(oh + 1) * 512]
                    if oh == 0:
                        nc.scalar.activation(half, po, AF.Copy)
                    else:
                        nc.vector.tensor_copy(half, po)
                    nc.sync.dma_start(out=out_d[tok, oh * 512:(oh + 1) * 512],
                                      in_=half)

            # chunk pipeline starts right after the nh0 (tokens 0:512)
            # projections; nh1 projections overlap the first B stages
            DEPTH = 2
            for c0 in range(DEPTH):
                stageA(c0)
                vtok_proj(c0)
            proj_group(0, 1)
            proj_group(1, 1)
            for c in range(NCH):
                stageB(c)
                if c + DEPTH < NCH:
                    stageA(c + DEPTH)
                    vtok_proj(c + DEPTH)
                stageB2(c)
                if c > 1:
                    out_block(c - 2)
            out_block(6)
            out_block(7)

    # Patch the act-table map so Exp and Ln both resolve to the combined
    # natural_log_exp_and_others set (otherwise the load-placement pass
    # alternates exp_and_others <-> natural_log per chunk, ~42us of reloads).
    import concourse.bacc as _bacc_mod
    from concourse.hw_specs import get_activation_tables as _gat
    _orig_gat = _bacc_mod.get_activation_tables

    def _patched_gat(arch):
        t = _gat(arch)
        for name, s in t.items():
            if name != "natural_log_exp_and_others":
                s.discard(AF.Exp)
                s.discard(AF.Ln)
        return t

    _bacc_mod.get_activation_tables = _patched_gat
    try:
        nc.compile()
    finally:
        _bacc_mod.get_activation_tables = _orig_gat
    return nc


_CACHE = {}


import os


def _get_nc():
    phase = int(os.environ.get("KPHASE", "3"))
    key = f"nc{phase}"
    if key not in _CACHE:
        _CACHE[key] = build_bass(phase)
    return _CACHE[key]


def _pf(w):  # (1024, ncol) -> (128, NF*ncol) bf16, [p, f, col]
    import ml_dtypes
    ncol = w.shape[1]
    return np.ascontiguousarray(
        w.reshape(NF, 128, ncol).transpose(1, 0, 2)).astype(ml_dtypes.bfloat16)


def _pT_pack(pb, bvc):  # p[b] (M, D), bv slice (E,) -> (128, (NF+4)*M) bf16
    import ml_dtypes
    out = np.zeros((128, NF + 4, M), np.float32)
    out[:, 0:NF, :] = pb.T.reshape(NF, 128, M).transpose(1, 0, 2)
    out[0, NF:NF + 4, :] = bvc.reshape(4, M)
    return np.ascontiguousarray(out).astype(ml_dtypes.bfloat16)


def make_in_maps(query, p, Wq, bq, Wpq, bpq, Wpc, bpc, Wk, bk, Wv, bv, Wo, bo):
    import ml_dtypes
    bf = ml_dtypes.bfloat16
    f32 = lambda a: np.ascontiguousarray(np.asarray(a), dtype=np.float32)
    query, p = f32(query), f32(p)
    Wq, Wpq, Wpc, Wk, Wv, Wo = map(f32, (Wq, Wpq, Wpc, Wk, Wv, Wo))
    bq, bpq, bpc, bk, bv, bo = map(f32, (bq, bpq, bpc, bk, bv, bo))
    rc = (1.0 / ((np.arange(N) + 1.0) * BETA)).astype(np.float32)
    rc_cols = np.ascontiguousarray(rc.reshape(NCH, 128).T)

    def col2(v):  # (256,) -> (128, 2)
        return np.ascontiguousarray(v.reshape(2, 128).T)

    in_maps = []
    for core in range(NCORES):
        b = core // 4
        hs = (core % 4) * HPC
        cols = slice(hs * DH, (hs + HPC) * DH)
        xT = query[b].T  # (D, N)
        w_et = {}
        for et in range(2):
            ecols = slice(hs * DH + et * 128, hs * DH + et * 128 + 128)
            # (128, NF, 3, 128): f-major so per-f granules are contiguous
            w_et[et] = np.stack([
                _pf((Wq[ecols, :] * SCALE).T).reshape(128, NF, 128),
                _pf(Wk[ecols, :].T).reshape(128, NF, 128),
                _pf(Wpc[ecols, :].T).reshape(128, NF, 128),
            ], axis=2)
        m = {
            "xta": _pf(np.ascontiguousarray(xT[:, 0:512])),
            "xtb": _pf(np.ascontiguousarray(xT[:, 512:1024])),
            "pT": _pT_pack(p[b], bv[cols]),
            "wpq": _pf((Wpq[cols, :] * SCALE).T),
            "wv": _pf(Wv[cols, :].T),
            "w0": np.ascontiguousarray(w_et[0]),
            "w1": np.ascontiguousarray(w_et[1]),
            "wo": np.ascontiguousarray(
                Wo[:, cols].T.reshape(2, 128, D).transpose(1, 0, 2)
            ).astype(bf),
            "aux": np.ascontiguousarray(np.concatenate([
                col2(bq[cols] * SCALE), col2(bk[cols]), col2(bpc[cols]),
                col2(bpq[cols] * SCALE), rc_cols,
                np.broadcast_to(bv[cols].reshape(1, E), (128, E)),
            ], axis=1, dtype=np.float32)),
        }
        in_maps.append(m)
    return in_maps


def kernel(query, p, dec_input_mask=None, p_mask=None,
           Wq=None, bq=None, Wpq=None, bpq=None, Wpc=None, bpc=None,
           Wk=None, bk=None, Wv=None, bv=None, Wo=None, bo=None,
           _trace=False, _trace_kwargs=None):
    in_maps = make_in_maps(query, p, Wq, bq, Wpq, bpq, Wpc, bpc,
                           Wk, bk, Wv, bv, Wo, bo)
    res = run_bass_kernel_spmd(_get_nc(), in_maps, core_ids=list(range(NCORES)),
                               trace=_trace, **(_trace_kwargs or {}))
    out = np.zeros((B, N, D), np.float32)
    for core in range(NCORES):
        out[core // 4] += res.results[core]["outp"].astype(np.float32)
    out += np.asarray(bo, np.float32).reshape(1, 1, D)
    if _trace:
        kernel.last_result = res
    return out
